# revision 1
# baseline (speedup 1.0000x reference)
"""Trainium2 Bass kernel for the 14-wire quantum autoencoder swap test.

Math reduction: reference wires 10-13 stay |0> until the swap test, so
P(aux=1) = (1 - q)/2 where q = sum_{i mod 8 == 0} |c_i|^2 over the 10-qubit
state c (wires 0-9) after AngleEmbedding + BasicEntanglerLayers.

Device layout (per core, 32 samples):
  state re/im tiles [128, 256] f32
  partition p = w9*64 + w8*32 + w7*16 + w6*8 + w5*4 + w4*2 + w3
  free      f = bh*128 + g*16 + bl   (b = bh*16+bl, g = w0*4 + w1*2 + w2)

The free axis splits into two independent half-batch streams (bh = 0/1) so
the DVE rotation phase of one half overlaps the PE matmul phase of the other.

Per entangler layer (gate order: RX all wires, then CNOT(w,w+1) w=0..9):
  - RX w0,w1 as tan-form scalar_tensor_tensor ops (cos deferred to the final
    affine), RX w2 fused with the pi = C12*C01 output permutation.
  - RX w3..w9 + C34..C89 as one host-built 128x128 complex matrix K2;
    C23 applied by using K2 on even-g columns and K2b = K2*X_w3 on odd-g
    columns (fp32 matmuls accumulating re/im in PSUM).
  - PSUM->SBUF copyback (ACT engine) folds C90: on w9=1 partitions g ^= 4.
Final: |.|^2 on partitions 0..15 (trash=000), per-sample reduce, ones-matmul
partition sum, affine 0.5 - 0.5*T^2*q.
"""
import numpy as np

NCORES = 8
B_CORE = 32
HB = 16            # half-batch
DEPTH = 4
NQ = 10

# packed const tile column layout
C_SCAL = 0         # [128p, 32]
C_SEED_RE = 32     # [32p, 32]
C_SEED_IM = 64
C_DBL_C = 96       # [32p, 2] (w8, w9)
C_DBL_S = 98
C_FIMN = 104       # [32p, 256]
C_FRE = 360
C_FIM = 616
C_TOT = 872

# ---------------------------------------------------------------------------
# Host-side plan construction
# ---------------------------------------------------------------------------


def _perm_matrix(perm):
    m = np.zeros((len(perm), len(perm)), dtype=np.float64)
    for src, dst in enumerate(perm):
        m[dst, src] = 1.0
    return m


def _cnot_chain_perm_p():
    perm = np.zeros(128, dtype=np.int64)
    for p in range(128):
        w = [(p >> k) & 1 for k in range(7)]
        for k in range(6):
            w[k + 1] ^= w[k]
        perm[p] = sum(w[k] << k for k in range(7))
    return perm


def _build_k2(weights_l):
    m = np.array([[1.0]], dtype=np.complex128)
    for w in (9, 8, 7, 6, 5, 4, 3):
        c, s = np.cos(weights_l[w] / 2), np.sin(weights_l[w] / 2)
        r = np.array([[c, -1j * s], [-1j * s, c]], dtype=np.complex128)
        m = np.kron(m, r)
    qa = _perm_matrix(_cnot_chain_perm_p())
    k2 = qa @ m
    k2b = k2 @ _perm_matrix(np.arange(128) ^ 1)
    return k2, k2b


def _make_shared(weights):
    """mats [128, 4*6*128] and the scal block, shared by all cores."""
    wt = weights.astype(np.float64).reshape(DEPTH, NQ)
    mats = np.zeros((128, DEPTH * 6 * 128), dtype=np.float32)
    scal = np.zeros((128, 32), dtype=np.float32)
    T = 1.0
    for l in range(DEPTH):
        k2, k2b = _build_k2(wt[l])
        blocks = [
            k2.real.T, (-k2.imag).T, k2.imag.T,
            k2b.real.T, (-k2b.imag).T, k2b.imag.T,
        ]
        for m_i, blk in enumerate(blocks):
            c0 = (l * 6 + m_i) * 128
            mats[:, c0:c0 + 128] = blk.astype(np.float32)
        for k, w in enumerate((0, 1, 2)):
            t = np.tan(wt[l, w] / 2)
            scal[:, l * 8 + 2 * k] = t
            scal[:, l * 8 + 2 * k + 1] = -t
            T *= np.cos(wt[l, w] / 2)
    scal[:, 31] = -0.5 * T * T
    return mats, scal


def _make_packed(features_core, scal):
    """Packed per-core const tensor [128, C_TOT]."""
    B = features_core.shape[0]
    th = features_core.astype(np.float64)
    c_emb, s_emb = np.cos(th / 2), np.sin(th / 2)
    v = np.stack([c_emb.astype(np.complex128), -1j * s_emb], axis=-1)

    # seed over wires 3..7: j = w7*16 + w6*8 + w5*4 + w4*2 + w3
    seed = np.empty((B, 32), dtype=np.complex128)
    for j in range(32):
        val = np.ones(B, dtype=np.complex128)
        for k, w in enumerate((3, 4, 5, 6, 7)):
            val = val * v[:, w, (j >> k) & 1]
        seed[:, j] = val

    F = np.empty((B, 8), dtype=np.complex128)
    for g in range(8):
        w0, w1, w2 = (g >> 2) & 1, (g >> 1) & 1, g & 1
        F[:, g] = v[:, 0, w0] * v[:, 1, w1] * v[:, 2, w2]
    # free col = bh*128 + g*16 + bl
    fbd = np.zeros((B, 8 * B), dtype=np.complex128)
    for b in range(B):
        bh, bl = divmod(b, HB)
        for g in range(8):
            fbd[b, bh * 128 + g * HB + bl] = F[b, g]

    packed = np.zeros((128, C_TOT), dtype=np.float32)
    packed[:, C_SCAL:C_SCAL + 32] = scal
    packed[0:B, C_SEED_RE:C_SEED_RE + 32] = seed.real
    packed[0:B, C_SEED_IM:C_SEED_IM + 32] = seed.imag
    packed[0:B, C_DBL_C] = c_emb[:, 8]
    packed[0:B, C_DBL_C + 1] = c_emb[:, 9]
    packed[0:B, C_DBL_S] = s_emb[:, 8]
    packed[0:B, C_DBL_S + 1] = s_emb[:, 9]
    packed[0:B, C_FIMN:C_FIMN + 256] = -fbd.imag
    packed[0:B, C_FRE:C_FRE + 256] = fbd.real
    packed[0:B, C_FIM:C_FIM + 256] = fbd.imag
    return packed


# ---------------------------------------------------------------------------
# Bass program
# ---------------------------------------------------------------------------

_PROGRAM = None


def _build_program(layer_reps=1):
    import concourse.bacc as bacc
    import concourse.mybir as mybir
    import concourse.tile as tile

    F32 = mybir.dt.float32
    MULT = mybir.AluOpType.mult
    ADD = mybir.AluOpType.add
    B = B_CORE

    nc = bacc.Bacc("TRN2", target_bir_lowering=False, debug=False,
                   num_devices=NCORES)

    d_pk = nc.dram_tensor("packed", [128, C_TOT], F32, kind="ExternalInput")
    d_mats = nc.dram_tensor("mats", [128, DEPTH * 6 * 128], F32,
                            kind="ExternalInput")
    d_out = nc.dram_tensor("out", [1, B], F32, kind="ExternalOutput")

    with tile.TileContext(nc) as tc:
        with (
            tc.tile_pool(name="const", bufs=1) as cpool,
            tc.tile_pool(name="state", bufs=10) as spool,
            tc.tile_pool(name="psum", bufs=6, space="PSUM") as ppool,
                                    tc.tile_pool(name="psumq", bufs=2, space="PSUM") as ppool_q,
        ):
            t_pk = cpool.tile([128, C_TOT], F32, tag="pk")
            t_mats = cpool.tile([128, DEPTH * 6 * 128], F32, tag="mats")
            t_ones = cpool.tile([16, 1], F32, tag="ones")
            t_wu = cpool.tile([128, 128], F32, tag="wu")

            # PE warm-up: junk matmuls to lift the clock gate while DMAs run
            nc.vector.memset(t_wu[:], 0.0)
            ps_wu = ppool_q.tile([128, 128], F32, tag="pq")
            for _ in range(7):
                nc.tensor.matmul(ps_wu[:], t_wu[:], t_wu[:],
                                 start=True, stop=True)

            nc.sync.dma_start(t_pk[0:B, 32:C_FIMN], d_pk[0:B, 32:C_FIMN])
            nc.sync.dma_start(t_pk[0:B, C_FIMN:], d_pk[0:B, C_FIMN:])
            nc.sync.dma_start(t_pk[:, 0:32], d_pk[:, 0:32])
            for l in range(DEPTH):
                c0 = l * 6 * 128
                nc.sync.dma_start(t_mats[:, c0:c0 + 768],
                                  d_mats[:, c0:c0 + 768])
            nc.vector.memset(t_ones[:], 1.0)

            def scal_ap(col, p=128):
                return t_pk[0:p, C_SCAL + col:C_SCAL + col + 1]

            # ---------------- embedding ----------------
            pt_re = spool.tile([B, 128], F32, tag="pt")
            pt_im = spool.tile([B, 128], F32, tag="pt")
            nc.vector.tensor_copy(pt_re[:, 0:32],
                                  t_pk[0:B, C_SEED_RE:C_SEED_RE + 32])
            nc.vector.tensor_copy(pt_im[:, 0:32],
                                  t_pk[0:B, C_SEED_IM:C_SEED_IM + 32])
            k = 32
            for j in range(2):  # wires 8, 9
                c_ap = t_pk[0:B, C_DBL_C + j:C_DBL_C + j + 1]
                s_ap = t_pk[0:B, C_DBL_S + j:C_DBL_S + j + 1]
                nc.vector.tensor_scalar(
                    pt_re[:, k:2 * k], pt_im[:, 0:k], s_ap, None, op0=MULT)
                nc.vector.tensor_scalar(
                    pt_im[:, k:2 * k], pt_re[:, 0:k], s_ap, -1.0,
                    op0=MULT, op1=MULT)
                nc.vector.tensor_scalar(
                    pt_re[:, 0:k], pt_re[:, 0:k], c_ap, None, op0=MULT)
                nc.vector.tensor_scalar(
                    pt_im[:, 0:k], pt_im[:, 0:k], c_ap, None, op0=MULT)
                k *= 2

            # S = PT.T @ Fbd, complex, stacked into one [128, 512] PSUM:
            #   psum = PTre @ [Fre | Fim] + PTim @ [Fimn | Fre] = [Sre | Sim]
            ps_s = ppool.tile([128, 512], F32, tag="ps")
            psv = ps_s[:].rearrange("p (i r) -> p i r", i=2, r=256)
            rhs1 = t_pk[0:B, C_FRE:C_FRE + 512].rearrange(
                "p (i r) -> p i r", i=2, r=256)
            rhs2 = t_pk[0:B, C_FIMN:C_FIMN + 512].rearrange(
                "p (i r) -> p i r", i=2, r=256)
            for hb in range(2):
                c0, c1 = hb * 128, hb * 128 + 128
                nc.tensor.matmul(psv[:, :, c0:c1], pt_re[:],
                                 rhs1[:, :, c0:c1], start=True, stop=False)
                nc.tensor.matmul(psv[:, :, c0:c1], pt_im[:],
                                 rhs2[:, :, c0:c1], start=False, stop=True)

            s_re = spool.tile([128, 8 * B], F32, tag="st")
            s_im = spool.tile([128, 8 * B], F32, tag="st")
            for hb in range(2):
                c0 = hb * 128
                nc.scalar.copy(s_re[:, c0:c0 + 128], ps_s[:, c0:c0 + 128])
                nc.scalar.copy(s_im[:, c0:c0 + 128],
                               ps_s[:, 256 + c0:256 + c0 + 128])

            # ---------------- entangler layers ----------------
            # per-half views (cols hb*128 .. hb*128+128): g-major, bl inner
            def half(t, hb, p0=0, p1=128):
                return t[p0:p1, hb * 128:hb * 128 + 128]

            def vi(t, hb):  # [p, 2 (w0), 64]
                return half(t, hb).rearrange("p (i r) -> p i r", i=2, r=64)

            def vu(t, hb, i):  # fixed w0 half -> [p, 2 (w1), 32]
                return half(t, hb).rearrange(
                    "p (i m r) -> p i m r", i=2, m=2, r=32)[:, i]

            def vq(t, hb, q):  # g-pair q -> [p, 2 (w2), 16]
                return half(t, hb).rearrange(
                    "p (q s b) -> p q s b", q=4, s=2, b=HB)[:, q]

            def vg(t, hb, p0=0, p1=128):  # [p, 8 (g), 16]
                return half(t, hb, p0, p1).rearrange(
                    "p (g b) -> p g b", g=8, b=HB)

            for rep in range(layer_reps):
              for l in range(DEPTH):
                is_last = rep == layer_reps - 1 and l == DEPTH - 1

                def tp(k):
                    return scal_ap(l * 8 + 2 * k)

                def tn(k):
                    return scal_ap(l * 8 + 2 * k + 1)

                a_re = spool.tile([128, 8 * B], F32, tag="st")
                a_im = spool.tile([128, 8 * B], F32, tag="st")
                b_re = spool.tile([128, 8 * B], F32, tag="st")
                b_im = spool.tile([128, 8 * B], F32, tag="st")
                c_re = spool.tile([128, 8 * B], F32, tag="st")
                c_im = spool.tile([128, 8 * B], F32, tag="st")
                pm_re = [None, None]
                pm_im = [None, None]

                for hb in range(2):
                    # R0: whole-half STT, w0 halves swapped on in0
                    nc.vector.scalar_tensor_tensor(
                        vi(a_re, hb), vi(s_im, hb)[:, ::-1, :], tp(0),
                        vi(s_re, hb), op0=MULT, op1=ADD)
                    nc.vector.scalar_tensor_tensor(
                        vi(a_im, hb), vi(s_re, hb)[:, ::-1, :], tn(0),
                        vi(s_im, hb), op0=MULT, op1=ADD)
                    # R1 per w0-half
                    for i in range(2):
                        nc.vector.scalar_tensor_tensor(
                            vu(b_re, hb, i), vu(a_im, hb, i)[:, ::-1, :],
                            tp(1), vu(a_re, hb, i), op0=MULT, op1=ADD)
                        nc.vector.scalar_tensor_tensor(
                            vu(b_im, hb, i), vu(a_re, hb, i)[:, ::-1, :],
                            tn(1), vu(a_im, hb, i), op0=MULT, op1=ADD)
                    # R2 + pi: out_q <- in1(b, maybe pair-swapped) + t2*in0
                    for (qo, qi, rev) in (
                        (0, 0, False), (1, 1, True), (2, 3, False),
                        (3, 2, True),
                    ):
                        for (dst, p1, p0, sc) in (
                            (c_re, b_re, b_im, tp(2)),
                            (c_im, b_im, b_re, tn(2)),
                        ):
                            if rev:
                                in1 = vq(p1, hb, qi)[:, ::-1, :]
                                in0 = vq(p0, hb, qi)
                            else:
                                in1 = vq(p1, hb, qi)
                                in0 = vq(p0, hb, qi)[:, ::-1, :]
                            nc.vector.scalar_tensor_tensor(
                                vq(dst, hb, qo), in0, sc, in1,
                                op0=MULT, op1=ADD)

                    # matmul: even g -> K2, odd g -> K2b
                    pm_re[hb] = ppool.tile([128, 128], F32, tag="ps", name=f"pmre{rep}_{l}{hb}")
                    pm_im[hb] = ppool.tile([128, 128], F32, tag="ps", name=f"pmim{rep}_{l}{hb}")

                    def mat(mi):
                        c0 = (l * 6 + mi) * 128
                        return t_mats[:, c0:c0 + 128]

                    pv_re = pm_re[hb][:].rearrange(
                        "p (g b) -> p g b", g=8, b=HB)
                    pv_im = pm_im[hb][:].rearrange(
                        "p (g b) -> p g b", g=8, b=HB)
                    for par, m0 in ((0, 0), (1, 3)):
                        xre = vg(c_re, hb)[:, par::2, :]
                        xim = vg(c_im, hb)[:, par::2, :]
                        nc.tensor.matmul(pv_re[:, par::2, :], mat(m0 + 0),
                                         xre, start=True, stop=False)
                        nc.tensor.matmul(pv_re[:, par::2, :], mat(m0 + 1),
                                         xim, start=False, stop=True)
                        nc.tensor.matmul(pv_im[:, par::2, :], mat(m0 + 2),
                                         xre, start=True, stop=False)
                        nc.tensor.matmul(pv_im[:, par::2, :], mat(m0 + 0),
                                         xim, start=False, stop=True)

                if not is_last:
                    s_re = spool.tile([128, 8 * B], F32, tag="st")
                    s_im = spool.tile([128, 8 * B], F32, tag="st")
                    for hb in range(2):
                        for (dst, src) in ((s_re, pm_re[hb]),
                                           (s_im, pm_im[hb])):
                            sv = src[:].rearrange("p (g b) -> p g b",
                                                  g=8, b=HB)
                            svh = src[:].rearrange("p (i r) -> p i r",
                                                   i=2, r=4 * HB)
                            # lower partitions: straight
                            nc.scalar.copy(vg(dst, hb, 0, 64), sv[0:64])
                            # upper: C90 fold (g ^= 4) = i-dim reversal
                            nc.scalar.copy(
                                half(dst, hb, 64, 128).rearrange(
                                    "p (i r) -> p i r", i=2, r=4 * HB),
                                svh[64:128, ::-1, :])
                else:
                    ss_re = [None, None]
                    ss_im = [None, None]
                    for hb in range(2):
                        ss_re[hb] = spool.tile([16, 128], F32, tag="fin", name=f"ssre{hb}")
                        ss_im[hb] = spool.tile([16, 128], F32, tag="fin", name=f"ssim{hb}")
                        nc.scalar.copy(ss_re[hb][:], pm_re[hb][0:16, :])
                        nc.scalar.copy(ss_im[hb][:], pm_im[hb][0:16, :])

            # ---------------- projection + output ----------------
            res = spool.tile([1, B], F32, tag="res")
            for hb in range(2):
                sq = spool.tile([16, 128], F32, tag="fin")
                sq2 = spool.tile([16, 128], F32, tag="fin")
                nc.vector.tensor_tensor(sq[:], ss_re[hb][:], ss_re[hb][:],
                                        op=MULT)
                nc.vector.tensor_tensor(sq2[:], ss_im[hb][:], ss_im[hb][:],
                                        op=MULT)
                nc.vector.tensor_tensor(sq[:], sq[:], sq2[:], op=ADD)
                q1 = spool.tile([16, HB], F32, tag="q1")
                nc.vector.tensor_reduce(
                    q1[:], sq[:].rearrange("p (g b) -> p b g", g=8, b=HB),
                    axis=mybir.AxisListType.X, op=ADD)
                pq = ppool_q.tile([1, HB], F32, tag="pq")
                nc.tensor.matmul(pq[:], t_ones[:], q1[:],
                                 start=True, stop=True)
                nc.vector.tensor_scalar(
                    res[:, hb * HB:hb * HB + HB], pq[:], scal_ap(31, 1),
                    0.5, op0=MULT, op1=ADD)
            nc.sync.dma_start(d_out[:], res[:])

    nc.compile()
    return nc


# ---------------------------------------------------------------------------
# Entry point
# ---------------------------------------------------------------------------


def kernel(features, weights):
    global _PROGRAM
    from concourse.bass_utils import run_bass_kernel_spmd

    features = np.asarray(features)
    weights = np.asarray(weights)
    if _PROGRAM is None:
        _PROGRAM = _build_program()
    nc = _PROGRAM

    mats, scal = _make_shared(weights)
    in_maps = []
    for c in range(NCORES):
        in_maps.append({
            "packed": _make_packed(
                features[c * B_CORE:(c + 1) * B_CORE], scal),
            "mats": mats,
        })

    # The NRT occasionally reports a transient "exec unit unrecoverable"
    # right after a prior process crashed; a fresh attempt succeeds.
    last_err = None
    for attempt in range(3):
        try:
            res = run_bass_kernel_spmd(nc, in_maps, list(range(NCORES)))
            break
        except Exception as e:  # noqa: BLE001
            last_err = e
            import time

            time.sleep(10 * (attempt + 1))
    else:
        raise last_err
    out = np.concatenate([res.results[c]["out"][0] for c in range(NCORES)])
    return out.astype(np.float32)


if __name__ == "__main__":
    rng = np.random.default_rng(0)
    f = rng.standard_normal((256, 10)).astype(np.float32)
    w = (0.01 * rng.random((4, 10))).astype(np.float32)
    print(kernel(f, w)[:8])



# revision 7
# speedup vs baseline: 1.2564x; 1.2564x over previous
"""Trainium2 Bass kernel for the 14-wire quantum autoencoder swap test.

Math: wires 10-13 stay |0> until the swap test, so
P(aux=1) = 0.5 - 0.5*q with q = sum_{trash wires 7,8,9 = 0} |c_i|^2 of the
10-qubit state after AngleEmbedding + BasicEntanglerLayers.

All transforms run on the PE in fp16 (fp32 PSUM accumulation):
  state S~ [ft, w] per 16-sample half-batch, ft = par*64 + w0*32 + w1*16 + bl
  (par = w2 after C01,C12; w = wires 3..9 index, w9 = MSB).
  Per layer: stage G (RX0-2 + C01 + C12 [+ C90 of previous layer folded via
  row-permuted GkC90 blocks for the w9=1 column half]) flips to standard
  layout [w, ft']; stage K (RX3-9 + C34..C89, C23 via K2/K2b stationary split
  by par class) flips back. Zero-padded stationary columns keep every matmul
  writing all 128 PSUM partitions (fp32-family dst-partition ISA rule).
  Negations live in host-built blocks: [Xim_neg | Xre | Xim] per matrix.
Final: |.|^2 on w<16 cols, sel-matmul partition reduce, affine.
"""
import numpy as np

NCORES = 8
B_CORE = 32
HB = 16
DEPTH = 4
NQ = 10

# pk16 fp16 [128, 832] column layout
C_SEED_RE = 0      # [32p, 32]
C_SEED_IM = 32
C_DBL = 64         # [32p, 4] = c8, s8, c9, s9
C_FSC = 128        # [16p, 512]: per half: [Fre 128 | Fim 128]
C_ZERO = 640       # [128p, 192] zeros
PK16 = 832

# pk32 f32 [128, 22]: sel [128,16], col16 = -0.5, cols 17:21 = c8,s8,c9,s9
PK32 = 22

# mats fp16 [128, 5760]: per layer: gR 384 | gC 384 (l>=1) | kR 384 | kB 384
L_COLS = [1152, 1536, 1536, 1536]
L_OFF = [0, 1152, 2688, 4224]
M_COLS = 5760

# ---------------------------------------------------------------------------
# Host-side constant construction
# ---------------------------------------------------------------------------

# ft class (par*4 + gg) -> g = w0*4 + w1*2 + w2
_FT_G = np.zeros(8, dtype=np.int64)
for _par in range(2):
    for _gg in range(4):
        _FT_G[_par * 4 + _gg] = (_gg >> 1) * 4 + (_gg & 1) * 2 + _par


def _perm_matrix(perm):
    m = np.zeros((len(perm), len(perm)))
    for src, dst in enumerate(perm):
        m[dst, src] = 1.0
    return m


def _cnot_chain_perm_p():
    perm = np.zeros(128, dtype=np.int64)
    for p in range(128):
        w = [(p >> k) & 1 for k in range(7)]
        for k in range(6):
            w[k + 1] ^= w[k]
        perm[p] = sum(w[k] << k for k in range(7))
    return perm


def _build_k2(weights_l):
    m = np.array([[1.0]], dtype=np.complex128)
    for w in (9, 8, 7, 6, 5, 4, 3):
        c, s = np.cos(weights_l[w] / 2), np.sin(weights_l[w] / 2)
        r = np.array([[c, -1j * s], [-1j * s, c]], dtype=np.complex128)
        m = np.kron(m, r)
    qa = _perm_matrix(_cnot_chain_perm_p())
    k2 = qa @ m
    k2b = k2 @ _perm_matrix(np.arange(128) ^ 1)
    return k2, k2b


def _build_gk(weights_l):
    m = np.array([[1.0]], dtype=np.complex128)
    for w in (0, 1, 2):
        c, s = np.cos(weights_l[w] / 2), np.sin(weights_l[w] / 2)
        r = np.array([[c, -1j * s], [-1j * s, c]], dtype=np.complex128)
        m = np.kron(m, r)
    perm = np.zeros(8, dtype=np.int64)
    for g in range(8):
        w0, w1, w2 = (g >> 2) & 1, (g >> 1) & 1, g & 1
        w1 ^= w0
        w2 ^= w1
        perm[g] = w0 * 4 + w1 * 2 + w2
    G = _perm_matrix(perm) @ m
    gk = np.zeros((128, 128), dtype=np.complex128)
    for ci in range(8):
        for co in range(8):
            v = G[_FT_G[co], _FT_G[ci]]
            if v != 0:
                for bl in range(HB):
                    gk[ci * 16 + bl, co * 16 + bl] = v
    return gk  # contraction: T[ft_out] = sum_ft_in S[ft_in] * gk[ft_in, ft_out]


def _blocks(m):
    """[im_neg | re | im] fp column triple for complex matrix m [128,128]."""
    return np.concatenate([-m.imag, m.real, m.imag], axis=1)


def _make_mats(weights):
    wt = weights.astype(np.float64).reshape(DEPTH, NQ)
    flip = np.arange(128) ^ 32
    cols = []
    for l in range(DEPTH):
        gk = _build_gk(wt[l])
        k2, k2b = _build_k2(wt[l])
        cols.append(_blocks(gk))
        if l >= 1:
            cols.append(_blocks(gk[flip, :]))
        cols.append(_blocks(k2.T))
        cols.append(_blocks(k2b.T))
    mats = np.concatenate(cols, axis=1)
    assert mats.shape == (128, M_COLS)
    return mats.astype(np.float16)


def _make_pk16(features_core):
    th = features_core.astype(np.float64)
    B = th.shape[0]
    c_emb, s_emb = np.cos(th / 2), np.sin(th / 2)
    v = np.stack([c_emb.astype(np.complex128), -1j * s_emb], axis=-1)

    pk = np.zeros((128, PK16), dtype=np.float64)
    # rows: sample b = hb*16+bl lives at partition hb*32+bl
    rows = np.array([(b // HB) * 32 + (b % HB) for b in range(B)])
    # seed over wires 3..7: col j = w7*16+w6*8+w5*4+w4*2+w3
    for j in range(32):
        val = np.ones(B, dtype=np.complex128)
        for k, w in enumerate((3, 4, 5, 6, 7)):
            val = val * v[:, w, (j >> k) & 1]
        pk[rows, C_SEED_RE + j] = val.real
        pk[rows, C_SEED_IM + j] = val.imag
    # fsc: per half: Fre/Fim [16, 128]; col ft = cls*16+bl, row hb*32+bl
    for hb in range(2):
        for cls in range(8):
            g = _FT_G[cls]
            w0, w1, w2 = (g >> 2) & 1, (g >> 1) & 1, g & 1
            for bl in range(HB):
                b = hb * HB + bl
                F = v[b, 0, w0] * v[b, 1, w1] * v[b, 2, w2]
                col = C_FSC + hb * 256
                pk[hb * 32 + bl, col + cls * 16 + bl] = F.real
                pk[hb * 32 + bl, col + 128 + cls * 16 + bl] = F.imag
    return pk.astype(np.float16)


def _make_pk32(features_core):
    th = features_core.astype(np.float64)
    B = th.shape[0]
    pk = np.zeros((128, PK32), dtype=np.float32)
    for ft in range(128):
        pk[ft, ft % 16] = 1.0
    pk[:, 16] = -0.5
    rows = np.array([(b // HB) * 32 + (b % HB) for b in range(B)])
    pk[rows, 17] = np.cos(th[:, 8] / 2)
    pk[rows, 18] = np.sin(th[:, 8] / 2)
    pk[rows, 19] = np.cos(th[:, 9] / 2)
    pk[rows, 20] = np.sin(th[:, 9] / 2)
    return pk


# ---------------------------------------------------------------------------
# Bass program
# ---------------------------------------------------------------------------

_PROGRAM = None


def _build_program():
    import concourse.bacc as bacc
    import concourse.mybir as mybir
    import concourse.tile as tile

    F32 = mybir.dt.float32
    F16 = mybir.dt.float16
    MULT = mybir.AluOpType.mult
    ADD = mybir.AluOpType.add

    nc = bacc.Bacc("TRN2", target_bir_lowering=False, debug=False,
                   num_devices=NCORES)

    d_pk16 = nc.dram_tensor("pk16", [128, PK16], F16, kind="ExternalInput")
    d_pk32 = nc.dram_tensor("pk32", [128, PK32], F32, kind="ExternalInput")
    d_mats = nc.dram_tensor("mats", [128, M_COLS], F16, kind="ExternalInput")
    d_out = nc.dram_tensor("out", [1, B_CORE], F32, kind="ExternalOutput")

    with tile.TileContext(nc) as tc:
        with (
            tc.tile_pool(name="const", bufs=1) as cpool,
            tc.tile_pool(name="psum", bufs=6, space="PSUM") as ppool,
            tc.tile_pool(name="psumq", bufs=2, space="PSUM") as qpool,
        ):
            t_pk16 = cpool.tile([128, PK16], F16, tag="pk16")
            t_pk32 = cpool.tile([128, PK32], F32, tag="pk32")
            t_mats = cpool.tile([128, M_COLS], F16, tag="mats")

            # DMAs in just-in-time order, single SP queue
            nc.sync.dma_start(t_pk16[:], d_pk16[:])
            nc.sync.dma_start(t_pk32[:], d_pk32[:])
            for l in range(DEPTH):
                sl = slice(L_OFF[l], L_OFF[l] + L_COLS[l])
                nc.sync.dma_start(t_mats[:, sl], d_mats[:, sl])

            # PE warm-up while DMAs run
            t_wu = cpool.tile([128, 256], F16, tag="wu")
            nc.vector.memset(t_wu[:], 0.0)
            ps_wu = ppool.tile([128, 256], F32, tag="ps", name="wu")
            for i in range(9):
                nc.tensor.matmul(ps_wu[:], t_wu[:, 0:128], t_wu[:],
                                 start=True, stop=True)

            # ---------------- embedding ----------------
            # pt [32, 384] fp16 = [imn | re | im]
            t_pt = cpool.tile([64, 384], F16, tag="pt")
            ptre = t_pt[0:48, 128:256]
            ptim = t_pt[0:48, 256:384]
            nc.vector.tensor_copy(ptre[:, 0:32],
                                  t_pk16[0:48, C_SEED_RE:C_SEED_RE + 32])
            nc.vector.tensor_copy(ptim[:, 0:32],
                                  t_pk16[0:48, C_SEED_IM:C_SEED_IM + 32])
            k = 32
            for j in range(2):  # wires 8, 9
                c_ap = t_pk32[0:48, 17 + 2 * j:18 + 2 * j]
                s_ap = t_pk32[0:48, 18 + 2 * j:19 + 2 * j]
                nc.vector.tensor_scalar(ptre[:, k:2 * k], ptim[:, 0:k],
                                        s_ap, None, op0=MULT)
                nc.vector.tensor_scalar(ptim[:, k:2 * k], ptre[:, 0:k],
                                        s_ap, -1.0, op0=MULT, op1=MULT)
                nc.vector.tensor_scalar(ptre[:, 0:k], ptre[:, 0:k],
                                        c_ap, None, op0=MULT)
                nc.vector.tensor_scalar(ptim[:, 0:k], ptim[:, 0:k],
                                        c_ap, None, op0=MULT)
                k *= 2
            nc.vector.tensor_scalar(t_pt[0:48, 0:128], ptim[:], -1.0, None,
                                    op0=MULT)

            # state tiles (persistent; zero-padded layout [128, 512]:
            # data chunks at 0,128,256,384 (64 cols), Z at 64,192,320)
            def zfill(t):
                dst = t[:].rearrange("p (a b) -> p a b", a=4, b=128)
                src = t_pk16[:, C_ZERO:C_ZERO + 192].rearrange(
                    "p (a b) -> p a b", a=3, b=64)
                nc.vector.tensor_copy(dst[:, 0:3, 64:128], src)

            s0 = []
            sAB = []
            tt = []
            for hb in range(2):
                s0.append(cpool.tile([128, 256], F16, tag=f"s0{hb}", name=f"s0{hb}"))
                a = cpool.tile([128, 512], F16, tag=f"sA{hb}", name=f"sA{hb}")
                b = cpool.tile([128, 512], F16, tag=f"sB{hb}", name=f"sB{hb}")
                t = cpool.tile([128, 512], F16, tag=f"tt{hb}", name=f"tt{hb}")
                for x in (a, b, t):
                    zfill(x)
                sAB.append([a, b])
                tt.append(t)

            # embedding matmuls -> psE -> s0
            for hb in range(2):
                psE = ppool.tile([128, 256], F32, tag="ps", name=f"pe{hb}")
                r0 = hb * 32
                fre = t_pk16[r0:r0 + HB, C_FSC + hb * 256:C_FSC + hb * 256 + 128]
                fim = t_pk16[r0:r0 + HB,
                             C_FSC + hb * 256 + 128:C_FSC + hb * 256 + 256]
                rows = slice(r0, r0 + HB)
                nc.tensor.matmul(psE[:], fre, t_pt[rows, 128:384],
                                 start=True, stop=False)
                nc.tensor.matmul(psE[:], fim, t_pt[rows, 0:256],
                                 start=False, stop=True)
                if hb == 0:
                    nc.vector.tensor_copy(s0[hb][:], psE[:])
                else:
                    nc.scalar.copy(s0[hb][:], psE[:])

            # ---------------- layers ----------------
            def mat(c0, c1):
                return t_mats[:, c0:c1]

            psK = [None, None]
            for l in range(DEPTH):
                base = L_OFF[l]
                gR = base
                gC = base + 384
                kR = base + (768 if l >= 1 else 384)
                kB = kR + 384
                for hb in range(2):
                    # stage G
                    psG = ppool.tile([128, 256], F32, tag="ps",
                                     name=f"pg{l}{hb}")
                    if l == 0:
                        nc.tensor.matmul(psG[:], s0[hb][:, 0:128],
                                         mat(gR + 128, gR + 384),
                                         start=True, stop=False)
                        nc.tensor.matmul(psG[:], s0[hb][:, 128:256],
                                         mat(gR, gR + 256),
                                         start=False, stop=True)
                    else:
                        s = sAB[hb][(l - 1) % 2]
                        nc.tensor.matmul(psG[:], s[:, 0:128],
                                         mat(gR + 128, gR + 384),
                                         start=True, stop=False)
                        nc.tensor.matmul(psG[:], s[:, 128:256],
                                         mat(gR, gR + 256),
                                         start=False, stop=False)
                        nc.tensor.matmul(psG[:], s[:, 192:320],
                                         mat(gC + 128, gC + 384),
                                         start=False, stop=False)
                        nc.tensor.matmul(psG[:], s[:, 320:448],
                                         mat(gC, gC + 256),
                                         start=False, stop=True)
                    # mid-copy psG -> t (4-chunk scatter)
                    tdst = tt[hb][:].rearrange("p (x y b) -> p x y b",
                                               x=2, y=2, b=128)[:, :, :, 0:64]
                    tsrc = psG[:].rearrange("p (ri par c) -> p par ri c",
                                            ri=2, par=2)
                    if (l + hb) % 2 == 0:
                        nc.vector.tensor_copy(tdst, tsrc)
                    else:
                        nc.scalar.copy(tdst, tsrc)
                    # stage K
                    psK[hb] = ppool.tile([128, 256], F32, tag="ps",
                                         name=f"pk{l}{hb}")
                    t = tt[hb]
                    nc.tensor.matmul(psK[hb][:], t[:, 0:128],
                                     mat(kR + 128, kR + 384),
                                     start=True, stop=False)
                    nc.tensor.matmul(psK[hb][:], t[:, 128:256],
                                     mat(kR, kR + 256),
                                     start=False, stop=False)
                    nc.tensor.matmul(psK[hb][:], t[:, 192:320],
                                     mat(kB + 128, kB + 384),
                                     start=False, stop=False)
                    nc.tensor.matmul(psK[hb][:], t[:, 320:448],
                                     mat(kB, kB + 256),
                                     start=False, stop=True)
                    if l < DEPTH - 1:
                        snext = sAB[hb][l % 2]
                        sdst = snext[:].rearrange(
                            "p (x y b) -> p x y b",
                            x=2, y=2, b=128)[:, :, :, 0:64]
                        ssrc = psK[hb][:].rearrange(
                            "p (ri w9 c) -> p w9 ri c", ri=2, w9=2)
                        if (l + hb) % 2 == 0:
                            nc.scalar.copy(sdst, ssrc)
                        else:
                            nc.vector.tensor_copy(sdst, ssrc)

            # ---------------- projection ----------------
            res2 = cpool.tile([16, 2], F32, tag="res")
            for hb in range(2):
                fin = cpool.tile([128, 32], F32, tag=f"fin{hb}",
                                 name=f"fin{hb}")
                v = psK[hb][:].rearrange("p (ri c) -> p ri c",
                                         ri=2)[:, :, 0:16]
                nc.vector.tensor_copy(
                    fin[:].rearrange("p (ri c) -> p ri c", ri=2), v)
                sq = cpool.tile([128, 32], F32, tag=f"sq{hb}", name=f"sq{hb}")
                nc.vector.tensor_tensor(sq[:], fin[:], fin[:], op=MULT)
                psq = qpool.tile([16, 32], F32, tag="pq", name=f"q{hb}")
                nc.tensor.matmul(psq[:], t_pk32[:, 0:16], sq[:],
                                 start=True, stop=True)
                q1 = cpool.tile([16, 1], F32, tag=f"q1{hb}", name=f"q1{hb}")
                nc.vector.tensor_reduce(q1[:], psq[:],
                                        axis=mybir.AxisListType.X, op=ADD)
                nc.vector.tensor_scalar(res2[:, hb:hb + 1], q1[:],
                                        t_pk32[0:16, 16:17], 0.5,
                                        op0=MULT, op1=ADD)
            dview = d_out[:].rearrange("o (hb bl) -> o bl hb", hb=2, bl=HB)
            nc.sync.dma_start(dview, res2[:])

    nc.compile()
    return nc


# ---------------------------------------------------------------------------
# Entry point
# ---------------------------------------------------------------------------


def kernel(features, weights):
    global _PROGRAM
    from concourse.bass_utils import run_bass_kernel_spmd

    features = np.asarray(features)
    weights = np.asarray(weights)
    if _PROGRAM is None:
        _PROGRAM = _build_program()
    nc = _PROGRAM

    mats = _make_mats(weights)
    in_maps = []
    for c in range(NCORES):
        fc = features[c * B_CORE:(c + 1) * B_CORE]
        in_maps.append({
            "pk16": _make_pk16(fc),
            "pk32": _make_pk32(fc),
            "mats": mats,
        })

    last_err = None
    for attempt in range(3):
        try:
            res = run_bass_kernel_spmd(nc, in_maps, list(range(NCORES)))
            break
        except Exception as e:  # noqa: BLE001
            last_err = e
            import time

            time.sleep(10 * (attempt + 1))
    else:
        raise last_err
    out = np.concatenate([res.results[c]["out"][0] for c in range(NCORES)])
    return out.astype(np.float32)


if __name__ == "__main__":
    rng = np.random.default_rng(0)
    f = rng.standard_normal((256, 10)).astype(np.float32)
    w = (0.01 * rng.random((4, 10))).astype(np.float32)
    print(kernel(f, w)[:8])


# revision 8
# speedup vs baseline: 1.3728x; 1.0927x over previous
"""Trainium2 Bass kernel for the 14-wire quantum autoencoder swap test.

Math: wires 10-13 stay |0> until the swap test, so
P(aux=1) = 0.5 - 0.5*q with q = sum_{trash wires 7,8,9 = 0} |c_i|^2 of the
10-qubit state after AngleEmbedding + BasicEntanglerLayers.

All transforms run on the PE in fp16 (fp32 PSUM accumulation):
  state S~ [ft, w] per 16-sample half-batch, ft = par*64 + w0*32 + w1*16 + bl
  (par = w2 after C01,C12; w = wires 3..9 index, w9 = MSB).
  Per layer: stage G (RX0-2 + C01 + C12 [+ C90 of previous layer folded via
  row-permuted GkC90 blocks for the w9=1 column half]) flips to standard
  layout [w, ft']; stage K (RX3-9 + C34..C89, C23 via K2/K2b stationary split
  by par class) flips back. Zero-padded stationary columns keep every matmul
  writing all 128 PSUM partitions (fp32-family dst-partition ISA rule).
  Negations live in host-built blocks: [Xim_neg | Xre | Xim] per matrix.
Final: |.|^2 on w<16 cols, sel-matmul partition reduce, affine.
"""
import numpy as np

NCORES = 8
B_CORE = 32
HB = 16
DEPTH = 4
NQ = 10

# pk16 fp16 [128, 1088]: fsc 512 | pt 384 ([imn|re|im]) | zeros 192
C_FSC = 0          # per half: [Fre 128 | Fim 128] at rows hb*32+bl
C_PT = 512
C_ZERO = 896
PK16 = 1088

# pk32 f32 [128, 18]: sel [128,16], col16 = -0.5
PK32 = 18

# mats fp16 [128, 5760]: per layer: gR 384 | gC 384 (l>=1) | kR 384 | kB 384
L_COLS = [1152, 1536, 1536, 1536]
L_OFF = [0, 1152, 2688, 4224]
M_COLS = 5760

# ---------------------------------------------------------------------------
# Host-side constant construction
# ---------------------------------------------------------------------------

# ft class (par*4 + gg) -> g = w0*4 + w1*2 + w2
_FT_G = np.zeros(8, dtype=np.int64)
for _par in range(2):
    for _gg in range(4):
        _FT_G[_par * 4 + _gg] = (_gg >> 1) * 4 + (_gg & 1) * 2 + _par


def _perm_matrix(perm):
    m = np.zeros((len(perm), len(perm)))
    for src, dst in enumerate(perm):
        m[dst, src] = 1.0
    return m


def _cnot_chain_perm_p():
    perm = np.zeros(128, dtype=np.int64)
    for p in range(128):
        w = [(p >> k) & 1 for k in range(7)]
        for k in range(6):
            w[k + 1] ^= w[k]
        perm[p] = sum(w[k] << k for k in range(7))
    return perm


def _build_k2(weights_l):
    m = np.array([[1.0]], dtype=np.complex128)
    for w in (9, 8, 7, 6, 5, 4, 3):
        c, s = np.cos(weights_l[w] / 2), np.sin(weights_l[w] / 2)
        r = np.array([[c, -1j * s], [-1j * s, c]], dtype=np.complex128)
        m = np.kron(m, r)
    qa = _perm_matrix(_cnot_chain_perm_p())
    k2 = qa @ m
    k2b = k2 @ _perm_matrix(np.arange(128) ^ 1)
    return k2, k2b


def _build_gk(weights_l):
    m = np.array([[1.0]], dtype=np.complex128)
    for w in (0, 1, 2):
        c, s = np.cos(weights_l[w] / 2), np.sin(weights_l[w] / 2)
        r = np.array([[c, -1j * s], [-1j * s, c]], dtype=np.complex128)
        m = np.kron(m, r)
    perm = np.zeros(8, dtype=np.int64)
    for g in range(8):
        w0, w1, w2 = (g >> 2) & 1, (g >> 1) & 1, g & 1
        w1 ^= w0
        w2 ^= w1
        perm[g] = w0 * 4 + w1 * 2 + w2
    G = _perm_matrix(perm) @ m
    gk = np.zeros((128, 128), dtype=np.complex128)
    for ci in range(8):
        for co in range(8):
            v = G[_FT_G[co], _FT_G[ci]]
            if v != 0:
                for bl in range(HB):
                    gk[ci * 16 + bl, co * 16 + bl] = v
    return gk  # contraction: T[ft_out] = sum_ft_in S[ft_in] * gk[ft_in, ft_out]


def _blocks(m):
    """[im_neg | re | im] fp column triple for complex matrix m [128,128]."""
    return np.concatenate([-m.imag, m.real, m.imag], axis=1)


def _make_mats(weights):
    wt = weights.astype(np.float64).reshape(DEPTH, NQ)
    flip = np.arange(128) ^ 32
    cols = []
    for l in range(DEPTH):
        gk = _build_gk(wt[l])
        k2, k2b = _build_k2(wt[l])
        cols.append(_blocks(gk))
        if l >= 1:
            cols.append(_blocks(gk[flip, :]))
        cols.append(_blocks(k2.T))
        cols.append(_blocks(k2b.T))
    mats = np.concatenate(cols, axis=1)
    assert mats.shape == (128, M_COLS)
    return mats.astype(np.float16)


def _make_pk16(features_core):
    th = features_core.astype(np.float64)
    B = th.shape[0]
    c_emb, s_emb = np.cos(th / 2), np.sin(th / 2)
    v = np.stack([c_emb.astype(np.complex128), -1j * s_emb], axis=-1)

    pk = np.zeros((128, PK16), dtype=np.float64)
    # rows: sample b = hb*16+bl lives at partition hb*32+bl
    rows = np.array([(b // HB) * 32 + (b % HB) for b in range(B)])
    # pt: seed over wires 3..9: col j = w9*64+...+w3 (w3 = LSB)
    for j in range(128):
        val = np.ones(B, dtype=np.complex128)
        for k, w in enumerate((3, 4, 5, 6, 7, 8, 9)):
            val = val * v[:, w, (j >> k) & 1]
        pk[rows, C_PT + j] = -val.imag
        pk[rows, C_PT + 128 + j] = val.real
        pk[rows, C_PT + 256 + j] = val.imag
    # fsc: per half: Fre/Fim [16, 128]; col ft = cls*16+bl, row hb*32+bl
    for hb in range(2):
        for cls in range(8):
            g = _FT_G[cls]
            w0, w1, w2 = (g >> 2) & 1, (g >> 1) & 1, g & 1
            for bl in range(HB):
                b = hb * HB + bl
                F = v[b, 0, w0] * v[b, 1, w1] * v[b, 2, w2]
                col = C_FSC + hb * 256
                pk[hb * 32 + bl, col + cls * 16 + bl] = F.real
                pk[hb * 32 + bl, col + 128 + cls * 16 + bl] = F.imag
    return pk.astype(np.float16)


def _make_pk32():
    pk = np.zeros((128, PK32), dtype=np.float32)
    for ft in range(128):
        pk[ft, ft % 16] = 1.0
    pk[:, 16] = -0.5
    return pk


# ---------------------------------------------------------------------------
# Bass program
# ---------------------------------------------------------------------------

_PROGRAM = None


def _build_program():
    import concourse.bacc as bacc
    import concourse.mybir as mybir
    import concourse.tile as tile

    F32 = mybir.dt.float32
    F16 = mybir.dt.float16
    MULT = mybir.AluOpType.mult
    ADD = mybir.AluOpType.add

    nc = bacc.Bacc("TRN2", target_bir_lowering=False, debug=False,
                   num_devices=NCORES)

    d_pk16 = nc.dram_tensor("pk16", [128, PK16], F16, kind="ExternalInput")
    d_pk32 = nc.dram_tensor("pk32", [128, PK32], F32, kind="ExternalInput")
    d_mats = nc.dram_tensor("mats", [128, M_COLS], F16, kind="ExternalInput")
    d_out = nc.dram_tensor("out", [1, B_CORE], F32, kind="ExternalOutput")

    with tile.TileContext(nc) as tc:
        with (
            tc.tile_pool(name="const", bufs=1) as cpool,
            tc.tile_pool(name="psum", bufs=6, space="PSUM") as ppool,
            tc.tile_pool(name="psumq", bufs=2, space="PSUM") as qpool,
        ):
            t_pk16 = cpool.tile([128, PK16], F16, tag="pk16")
            t_pk32 = cpool.tile([128, PK32], F32, tag="pk32")
            t_mats = cpool.tile([128, M_COLS], F16, tag="mats")

            # DMAs in just-in-time order, single SP queue
            nc.sync.dma_start(t_pk16[:], d_pk16[:])
            nc.sync.dma_start(t_pk32[:], d_pk32[:])
            for l in range(DEPTH):
                sl = slice(L_OFF[l], L_OFF[l] + L_COLS[l])
                nc.sync.dma_start(t_mats[:, sl], d_mats[:, sl])

            # PE warm-up while DMAs run
            t_wu = cpool.tile([128, 256], F16, tag="wu")
            nc.gpsimd.memset(t_wu[:], 0.0)
            ps_wu = ppool.tile([128, 256], F32, tag="ps", name="wu")
            for i in range(10):
                nc.tensor.matmul(ps_wu[:], t_wu[:, 0:128], t_wu[:],
                                 start=True, stop=True)

            # state tiles (persistent; zero-padded layout [128, 512]:
            # data chunks at 0,128,256,384 (64 cols), Z at 64,192,320)
            def zfill(t):
                dst = t[:].rearrange("p (a b) -> p a b", a=4, b=128)
                src = t_pk16[:, C_ZERO:C_ZERO + 192].rearrange(
                    "p (a b) -> p a b", a=3, b=64)
                nc.vector.tensor_copy(dst[:, 0:3, 64:128], src)

            s0 = []
            sAB = []
            tt = []
            for hb in range(2):
                s0.append(cpool.tile([128, 256], F16, tag=f"s0{hb}", name=f"s0{hb}"))
                a = cpool.tile([128, 512], F16, tag=f"sA{hb}", name=f"sA{hb}")
                b = cpool.tile([128, 512], F16, tag=f"sB{hb}", name=f"sB{hb}")
                t = cpool.tile([128, 512], F16, tag=f"tt{hb}", name=f"tt{hb}")
                for x in (a, b, t):
                    zfill(x)
                sAB.append([a, b])
                tt.append(t)

            # embedding matmuls -> psE -> s0 (split copies re/im)
            for hb in range(2):
                psE = ppool.tile([128, 256], F32, tag="ps", name=f"pe{hb}")
                r0 = hb * 32
                fre = t_pk16[r0:r0 + HB, C_FSC + hb * 256:C_FSC + hb * 256 + 128]
                fim = t_pk16[r0:r0 + HB,
                             C_FSC + hb * 256 + 128:C_FSC + hb * 256 + 256]
                rows = slice(r0, r0 + HB)
                nc.tensor.matmul(psE[:], fre,
                                 t_pk16[rows, C_PT + 128:C_PT + 384],
                                 start=True, stop=False)
                nc.tensor.matmul(psE[:], fim,
                                 t_pk16[rows, C_PT:C_PT + 256],
                                 start=False, stop=True)
                nc.scalar.copy(s0[hb][:, 0:128], psE[:, 0:128])
                nc.vector.tensor_copy(s0[hb][:, 128:256], psE[:, 128:256])

            # ---------------- layers ----------------
            def mat(c0, c1):
                return t_mats[:, c0:c1]

            psK = [None, None]
            for l in range(DEPTH):
                base = L_OFF[l]
                gR = base
                gC = base + 384
                kR = base + (768 if l >= 1 else 384)
                kB = kR + 384
                for hb in range(2):
                    # stage G
                    psG = ppool.tile([128, 256], F32, tag="ps",
                                     name=f"pg{l}{hb}")
                    if l == 0:
                        nc.tensor.matmul(psG[:], s0[hb][:, 0:128],
                                         mat(gR + 128, gR + 384),
                                         start=True, stop=False)
                        nc.tensor.matmul(psG[:], s0[hb][:, 128:256],
                                         mat(gR, gR + 256),
                                         start=False, stop=True)
                    else:
                        s = sAB[hb][(l - 1) % 2]
                        nc.tensor.matmul(psG[:], s[:, 0:128],
                                         mat(gR + 128, gR + 384),
                                         start=True, stop=False)
                        nc.tensor.matmul(psG[:], s[:, 128:256],
                                         mat(gR, gR + 256),
                                         start=False, stop=False)
                        nc.tensor.matmul(psG[:], s[:, 192:320],
                                         mat(gC + 128, gC + 384),
                                         start=False, stop=False)
                        nc.tensor.matmul(psG[:], s[:, 320:448],
                                         mat(gC, gC + 256),
                                         start=False, stop=True)
                    # mid-copy psG -> t, split ev/od on ACT + DVE
                    tdst = tt[hb][:].rearrange("p (x y b) -> p x y b",
                                               x=2, y=2, b=128)[:, :, :, 0:64]
                    tsrc = psG[:].rearrange("p (ri par c) -> p ri par c",
                                            ri=2, par=2)
                    nc.scalar.copy(tdst[:, 0], tsrc[:, :, 0])
                    nc.vector.tensor_copy(tdst[:, 1], tsrc[:, :, 1])
                    # stage K
                    psK[hb] = ppool.tile([128, 256], F32, tag="ps",
                                         name=f"pk{l}{hb}")
                    t = tt[hb]
                    nc.tensor.matmul(psK[hb][:], t[:, 0:128],
                                     mat(kR + 128, kR + 384),
                                     start=True, stop=False)
                    nc.tensor.matmul(psK[hb][:], t[:, 128:256],
                                     mat(kR, kR + 256),
                                     start=False, stop=False)
                    nc.tensor.matmul(psK[hb][:], t[:, 192:320],
                                     mat(kB + 128, kB + 384),
                                     start=False, stop=False)
                    nc.tensor.matmul(psK[hb][:], t[:, 320:448],
                                     mat(kB, kB + 256),
                                     start=False, stop=True)
                    if l < DEPTH - 1:
                        snext = sAB[hb][l % 2]
                        sdst = snext[:].rearrange(
                            "p (x y b) -> p x y b",
                            x=2, y=2, b=128)[:, :, :, 0:64]
                        ssrc = psK[hb][:].rearrange(
                            "p (ri w9 c) -> p ri w9 c", ri=2, w9=2)
                        nc.scalar.copy(sdst[:, 0], ssrc[:, :, 0])
                        nc.vector.tensor_copy(sdst[:, 1], ssrc[:, :, 1])

            # ---------------- projection ----------------
            res2 = cpool.tile([16, 2], F32, tag="res")
            SQUARE = mybir.ActivationFunctionType.Square
            for hb in range(2):
                v = psK[hb][:].rearrange("p (ri c) -> p ri c",
                                         ri=2)[:, :, 0:16]
                sq = cpool.tile([128, 32], F32, tag=f"sq{hb}", name=f"sq{hb}")
                nc.scalar.activation(
                    sq[:].rearrange("p (ri c) -> p ri c", ri=2), v, SQUARE)
                psq = qpool.tile([16, 32], F32, tag="pq", name=f"q{hb}")
                nc.tensor.matmul(psq[:], t_pk32[:, 0:16], sq[:],
                                 start=True, stop=True)
                q1 = cpool.tile([16, 1], F32, tag=f"q1{hb}", name=f"q1{hb}")
                nc.vector.tensor_reduce(q1[:], psq[:],
                                        axis=mybir.AxisListType.X, op=ADD)
                nc.vector.tensor_scalar(res2[:, hb:hb + 1], q1[:],
                                        t_pk32[0:16, 16:17], 0.5,
                                        op0=MULT, op1=ADD)
                nc.sync.dma_start(d_out[:, hb * HB:hb * HB + HB],
                                  res2[:, hb:hb + 1])

    nc.compile()
    return nc


# ---------------------------------------------------------------------------
# Entry point
# ---------------------------------------------------------------------------


def kernel(features, weights):
    global _PROGRAM
    from concourse.bass_utils import run_bass_kernel_spmd

    features = np.asarray(features)
    weights = np.asarray(weights)
    if _PROGRAM is None:
        _PROGRAM = _build_program()
    nc = _PROGRAM

    mats = _make_mats(weights)
    pk32 = _make_pk32()
    in_maps = []
    for c in range(NCORES):
        fc = features[c * B_CORE:(c + 1) * B_CORE]
        in_maps.append({
            "pk16": _make_pk16(fc),
            "pk32": pk32,
            "mats": mats,
        })

    last_err = None
    for attempt in range(3):
        try:
            res = run_bass_kernel_spmd(nc, in_maps, list(range(NCORES)))
            break
        except Exception as e:  # noqa: BLE001
            last_err = e
            import time

            time.sleep(10 * (attempt + 1))
    else:
        raise last_err
    out = np.concatenate([res.results[c]["out"][0] for c in range(NCORES)])
    return out.astype(np.float32)


if __name__ == "__main__":
    rng = np.random.default_rng(0)
    f = rng.standard_normal((256, 10)).astype(np.float32)
    w = (0.01 * rng.random((4, 10))).astype(np.float32)
    print(kernel(f, w)[:8])


# revision 10
# speedup vs baseline: 1.4872x; 1.0833x over previous
"""Trainium2 Bass kernel for the 14-wire quantum autoencoder swap test.

Math: wires 10-13 stay |0> until the swap test, so
P(aux=1) = 0.5 - 0.5*q with q = sum_{trash wires 7,8,9 = 0} |c_i|^2 of the
10-qubit state after AngleEmbedding + BasicEntanglerLayers.

All transforms run on the PE in fp16 (fp32 PSUM accumulation):
  state S~ [ft, w] per 16-sample half-batch, ft = par*64 + w0*32 + w1*16 + bl
  (par = w2 after C01,C12; w = wires 3..9 index, w9 = MSB).
  Per layer: stage G (RX0-2 + C01 + C12 [+ C90 of previous layer folded via
  row-permuted GkC90 blocks for the w9=1 column half]) flips to standard
  layout [w, ft']; stage K (RX3-9 + C34..C89, C23 via K2/K2b stationary split
  by par class) flips back. Zero-padded stationary columns keep every matmul
  writing all 128 PSUM partitions (fp32-family dst-partition ISA rule).
  Negations live in host-built blocks: [Xim_neg | Xre | Xim] per matrix.
Final: |.|^2 on w<16 cols, sel-matmul partition reduce, affine.
"""
import numpy as np

NCORES = 8
B_CORE = 32
HB = 16
DEPTH = 4
NQ = 10

# pk16 fp16 [128, 1088]: fsc 512 | pt 384 ([imn|re|im]) | zeros 192
C_FSC = 0          # per half: [Fre 128 | Fim 128] at rows hb*32+bl
C_PT = 512
C_ZERO = 896
PK16 = 1088

# pk32 f32 [128, 18]: sel [128,16], col16 = -0.5
PK32 = 18

# mats fp16 [128, 5760]: per layer: gR 384 | gC 384 (l>=1) | kR 384 | kB 384
L_COLS = [1152, 1536, 1536, 1536]
L_OFF = [0, 1152, 2688, 4224]
M_COLS = 5760

# ---------------------------------------------------------------------------
# Host-side constant construction
# ---------------------------------------------------------------------------

# ft class (par*4 + gg) -> g = w0*4 + w1*2 + w2
_FT_G = np.zeros(8, dtype=np.int64)
for _par in range(2):
    for _gg in range(4):
        _FT_G[_par * 4 + _gg] = (_gg >> 1) * 4 + (_gg & 1) * 2 + _par


def _perm_matrix(perm):
    m = np.zeros((len(perm), len(perm)))
    for src, dst in enumerate(perm):
        m[dst, src] = 1.0
    return m


def _cnot_chain_perm_p():
    perm = np.zeros(128, dtype=np.int64)
    for p in range(128):
        w = [(p >> k) & 1 for k in range(7)]
        for k in range(6):
            w[k + 1] ^= w[k]
        perm[p] = sum(w[k] << k for k in range(7))
    return perm


def _build_k2(weights_l):
    m = np.array([[1.0]], dtype=np.complex128)
    for w in (9, 8, 7, 6, 5, 4, 3):
        c, s = np.cos(weights_l[w] / 2), np.sin(weights_l[w] / 2)
        r = np.array([[c, -1j * s], [-1j * s, c]], dtype=np.complex128)
        m = np.kron(m, r)
    qa = _perm_matrix(_cnot_chain_perm_p())
    k2 = qa @ m
    k2b = k2 @ _perm_matrix(np.arange(128) ^ 1)
    return k2, k2b


def _build_gk(weights_l):
    m = np.array([[1.0]], dtype=np.complex128)
    for w in (0, 1, 2):
        c, s = np.cos(weights_l[w] / 2), np.sin(weights_l[w] / 2)
        r = np.array([[c, -1j * s], [-1j * s, c]], dtype=np.complex128)
        m = np.kron(m, r)
    perm = np.zeros(8, dtype=np.int64)
    for g in range(8):
        w0, w1, w2 = (g >> 2) & 1, (g >> 1) & 1, g & 1
        w1 ^= w0
        w2 ^= w1
        perm[g] = w0 * 4 + w1 * 2 + w2
    G = _perm_matrix(perm) @ m
    gk = np.zeros((128, 128), dtype=np.complex128)
    for ci in range(8):
        for co in range(8):
            v = G[_FT_G[co], _FT_G[ci]]
            if v != 0:
                for bl in range(HB):
                    gk[ci * 16 + bl, co * 16 + bl] = v
    return gk  # contraction: T[ft_out] = sum_ft_in S[ft_in] * gk[ft_in, ft_out]


def _blocks(m):
    """[im_neg | re | im] fp column triple for complex matrix m [128,128]."""
    return np.concatenate([-m.imag, m.real, m.imag], axis=1)


def _make_mats(weights):
    wt = weights.astype(np.float64).reshape(DEPTH, NQ)
    flip = np.arange(128) ^ 32
    cols = []
    for l in range(DEPTH):
        gk = _build_gk(wt[l])
        k2, k2b = _build_k2(wt[l])
        cols.append(_blocks(gk))
        if l >= 1:
            cols.append(_blocks(gk[flip, :]))
        cols.append(_blocks(k2.T))
        cols.append(_blocks(k2b.T))
    mats = np.concatenate(cols, axis=1)
    assert mats.shape == (128, M_COLS)
    return mats.astype(np.float16)


def _make_pk16(features_core):
    th = features_core.astype(np.float64)
    B = th.shape[0]
    c_emb, s_emb = np.cos(th / 2), np.sin(th / 2)
    v = np.stack([c_emb.astype(np.complex128), -1j * s_emb], axis=-1)

    pk = np.zeros((128, PK16), dtype=np.float64)
    # rows: sample b = hb*16+bl lives at partition hb*32+bl
    rows = np.array([(b // HB) * 32 + (b % HB) for b in range(B)])
    # pt: seed over wires 3..9: col j = w9*64+...+w3 (w3 = LSB)
    for j in range(128):
        val = np.ones(B, dtype=np.complex128)
        for k, w in enumerate((3, 4, 5, 6, 7, 8, 9)):
            val = val * v[:, w, (j >> k) & 1]
        pk[rows, C_PT + j] = -val.imag
        pk[rows, C_PT + 128 + j] = val.real
        pk[rows, C_PT + 256 + j] = val.imag
    # fsc: per half: Fre/Fim [16, 128]; col ft = cls*16+bl, row hb*32+bl
    for hb in range(2):
        for cls in range(8):
            g = _FT_G[cls]
            w0, w1, w2 = (g >> 2) & 1, (g >> 1) & 1, g & 1
            for bl in range(HB):
                b = hb * HB + bl
                F = v[b, 0, w0] * v[b, 1, w1] * v[b, 2, w2]
                col = C_FSC + hb * 256
                pk[hb * 32 + bl, col + cls * 16 + bl] = F.real
                pk[hb * 32 + bl, col + 128 + cls * 16 + bl] = F.imag
    return pk.astype(np.float16)


def _make_pk32():
    pk = np.zeros((128, PK32), dtype=np.float32)
    for ft in range(128):
        pk[ft, ft % 16] = 1.0
    pk[:, 16] = -0.5
    return pk


# ---------------------------------------------------------------------------
# Bass program
# ---------------------------------------------------------------------------

_PROGRAM = None


def _build_program():
    import concourse.bacc as bacc
    import concourse.mybir as mybir
    import concourse.tile as tile

    F32 = mybir.dt.float32
    F16 = mybir.dt.float16
    MULT = mybir.AluOpType.mult
    ADD = mybir.AluOpType.add

    nc = bacc.Bacc("TRN2", target_bir_lowering=False, debug=False,
                   num_devices=NCORES)

    d_pk16 = nc.dram_tensor("pk16", [128, PK16], F16, kind="ExternalInput")
    d_pk32 = nc.dram_tensor("pk32", [128, PK32], F32, kind="ExternalInput")
    d_mats = nc.dram_tensor("mats", [128, M_COLS], F16, kind="ExternalInput")
    d_out = nc.dram_tensor("out", [1, B_CORE], F32, kind="ExternalOutput")

    with tile.TileContext(nc) as tc:
        with (
            tc.tile_pool(name="const", bufs=1) as cpool,
            tc.tile_pool(name="psum", bufs=6, space="PSUM") as ppool,
            tc.tile_pool(name="psumb", bufs=2, space="PSUM") as bpool,
        ):
            t_pk16 = cpool.tile([128, PK16], F16, tag="pk16")
            t_pk32 = cpool.tile([128, PK32], F32, tag="pk32")
            t_mats = cpool.tile([128, M_COLS], F16, tag="mats")

            # DMAs in just-in-time order, single SP queue
            nc.sync.dma_start(t_pk16[0:64, 0:C_ZERO], d_pk16[0:64, 0:C_ZERO])
            nc.sync.dma_start(t_pk16[:, C_ZERO:PK16], d_pk16[:, C_ZERO:PK16])
            sl = slice(L_OFF[0], L_OFF[0] + L_COLS[0])
            nc.sync.dma_start(t_mats[:, sl], d_mats[:, sl])
            nc.sync.dma_start(t_pk32[:], d_pk32[:])
            for l in range(1, DEPTH):
                sl = slice(L_OFF[l], L_OFF[l] + L_COLS[l])
                nc.sync.dma_start(t_mats[:, sl], d_mats[:, sl])

            # PE warm-up: starts the PE ramp clock early
            t_wu = cpool.tile([128, 256], F16, tag="wu")
            nc.gpsimd.memset(t_wu[:], 0.0)
            ps_wu = ppool.tile([128, 128], F32, tag="ps", name="wu")
            for i in range(3):
                nc.tensor.matmul(ps_wu[:], t_wu[:, 0:128], t_wu[:, 0:128],
                                 start=True, stop=True)

            # state tiles (persistent; zero-padded layout [128, 512]:
            # data chunks at 0,128,256,384 (64 cols), Z at 64,192,320)
            def zfill(t):
                dst = t[:].rearrange("p (a b) -> p a b", a=4, b=128)
                src = t_pk16[:, C_ZERO:C_ZERO + 192].rearrange(
                    "p (a b) -> p a b", a=3, b=64)
                nc.vector.tensor_copy(dst[:, 0:3, 64:128], src)

            s0 = []
            sAB = []
            tt = []
            for hb in range(2):
                s0.append(cpool.tile([128, 256], F16, tag=f"s0{hb}",
                                     name=f"s0{hb}"))
                a = cpool.tile([128, 512], F16, tag=f"sA{hb}", name=f"sA{hb}")
                b = cpool.tile([128, 512], F16, tag=f"sB{hb}", name=f"sB{hb}")
                t = cpool.tile([128, 512], F16, tag=f"tt{hb}", name=f"tt{hb}")
                for x in (a, b, t):
                    zfill(x)
                sAB.append([a, b])
                tt.append(t)

            # chunk views of a zero-padded tile: [p, x(par/w9), y(ri), 64]
            def chunks(t):
                return t[:].rearrange("p (x y b) -> p x y b",
                                      x=2, y=2, b=128)[:, :, :, 0:64]

            # embedding matmuls (re/im split) -> psE -> s0
            for hb in range(2):
                r0 = hb * 32
                rows = slice(r0, r0 + HB)
                fre = t_pk16[r0:r0 + HB, C_FSC + hb * 256:C_FSC + hb * 256 + 128]
                fim = t_pk16[r0:r0 + HB,
                             C_FSC + hb * 256 + 128:C_FSC + hb * 256 + 256]
                ptimn = t_pk16[rows, C_PT:C_PT + 128]
                ptre = t_pk16[rows, C_PT + 128:C_PT + 256]
                ptim = t_pk16[rows, C_PT + 256:C_PT + 384]
                psEr = ppool.tile([128, 128], F32, tag="ps", name=f"per{hb}")
                psEi = ppool.tile([128, 128], F32, tag="ps", name=f"pei{hb}")
                nc.tensor.matmul(psEr[:], fre, ptre, start=True, stop=False)
                nc.tensor.matmul(psEr[:], fim, ptimn, start=False, stop=True)
                nc.tensor.matmul(psEi[:], fre, ptim, start=True, stop=False)
                nc.tensor.matmul(psEi[:], fim, ptre, start=False, stop=True)
                nc.scalar.copy(s0[hb][:, 0:128], psEr[:])
                nc.vector.tensor_copy(s0[hb][:, 128:256], psEi[:])

            # ---------------- layers ----------------
            def mat(c0, c1):
                return t_mats[:, c0:c1]

            psK3 = [None, None]
            for l in range(DEPTH):
                base = L_OFF[l]
                gR = base
                gC = base + 384
                kR = base + (768 if l >= 1 else 384)
                kB = kR + 384
                for hb in range(2):
                    # ---- stage G: re/im split psums, 128-col MMs ----
                    pgr = ppool.tile([128, 128], F32, tag="ps",
                                     name=f"pgr{l}{hb}")
                    pgi = ppool.tile([128, 128], F32, tag="ps",
                                     name=f"pgi{l}{hb}")
                    if l == 0:
                        sre = s0[hb][:, 0:128]
                        sim = s0[hb][:, 128:256]
                        nc.tensor.matmul(pgr[:], sre, mat(gR + 128, gR + 256),
                                         start=True, stop=False)
                        nc.tensor.matmul(pgr[:], sim, mat(gR, gR + 128),
                                         start=False, stop=True)
                        nc.tensor.matmul(pgi[:], sre, mat(gR + 256, gR + 384),
                                         start=True, stop=False)
                        nc.tensor.matmul(pgi[:], sim, mat(gR + 128, gR + 256),
                                         start=False, stop=True)
                    else:
                        s = sAB[hb][(l - 1) % 2]
                        # order: re-gated chunks first, then im-gated
                        nc.tensor.matmul(pgr[:], s[:, 0:128],
                                         mat(gR + 128, gR + 256),
                                         start=True, stop=False)
                        nc.tensor.matmul(pgr[:], s[:, 192:320],
                                         mat(gC + 128, gC + 256),
                                         start=False, stop=False)
                        nc.tensor.matmul(pgr[:], s[:, 128:256],
                                         mat(gR, gR + 128),
                                         start=False, stop=False)
                        nc.tensor.matmul(pgr[:], s[:, 320:448],
                                         mat(gC, gC + 128),
                                         start=False, stop=True)
                        nc.tensor.matmul(pgi[:], s[:, 0:128],
                                         mat(gR + 256, gR + 384),
                                         start=True, stop=False)
                        nc.tensor.matmul(pgi[:], s[:, 192:320],
                                         mat(gC + 256, gC + 384),
                                         start=False, stop=False)
                        nc.tensor.matmul(pgi[:], s[:, 128:256],
                                         mat(gR + 128, gR + 256),
                                         start=False, stop=False)
                        nc.tensor.matmul(pgi[:], s[:, 320:448],
                                         mat(gC + 128, gC + 256),
                                         start=False, stop=True)
                    # mid-copies: re -> ACT, im -> DVE (parallel)
                    tch = chunks(tt[hb])
                    nc.scalar.copy(
                        tch[:, :, 0],
                        pgr[:].rearrange("p (par c) -> p par c", par=2))
                    nc.vector.tensor_copy(
                        tch[:, :, 1],
                        pgi[:].rearrange("p (par c) -> p par c", par=2))

                    # ---- stage K ----
                    t = tt[hb]
                    if l < DEPTH - 1:
                        pkr = ppool.tile([128, 128], F32, tag="ps",
                                         name=f"pkr{l}{hb}")
                        pki = ppool.tile([128, 128], F32, tag="ps",
                                         name=f"pki{l}{hb}")
                        nc.tensor.matmul(pkr[:], t[:, 0:128],
                                         mat(kR + 128, kR + 256),
                                         start=True, stop=False)
                        nc.tensor.matmul(pkr[:], t[:, 192:320],
                                         mat(kB + 128, kB + 256),
                                         start=False, stop=False)
                        nc.tensor.matmul(pkr[:], t[:, 128:256],
                                         mat(kR, kR + 128),
                                         start=False, stop=False)
                        nc.tensor.matmul(pkr[:], t[:, 320:448],
                                         mat(kB, kB + 128),
                                         start=False, stop=True)
                        nc.tensor.matmul(pki[:], t[:, 0:128],
                                         mat(kR + 256, kR + 384),
                                         start=True, stop=False)
                        nc.tensor.matmul(pki[:], t[:, 192:320],
                                         mat(kB + 256, kB + 384),
                                         start=False, stop=False)
                        nc.tensor.matmul(pki[:], t[:, 128:256],
                                         mat(kR + 128, kR + 256),
                                         start=False, stop=False)
                        nc.tensor.matmul(pki[:], t[:, 320:448],
                                         mat(kB + 128, kB + 256),
                                         start=False, stop=True)
                        snext = sAB[hb][l % 2]
                        sch = chunks(snext)
                        nc.scalar.copy(
                            sch[:, :, 0],
                            pkr[:].rearrange("p (w9 c) -> p w9 c", w9=2))
                        nc.vector.tensor_copy(
                            sch[:, :, 1],
                            pki[:].rearrange("p (w9 c) -> p w9 c", w9=2))
                    else:
                        # last layer: unsplit 256-col psum for the finale
                        psK3[hb] = bpool.tile([128, 256], F32, tag="pb",
                                              name=f"pk3{hb}")
                        nc.tensor.matmul(psK3[hb][:], t[:, 0:128],
                                         mat(kR + 128, kR + 384),
                                         start=True, stop=False)
                        nc.tensor.matmul(psK3[hb][:], t[:, 192:320],
                                         mat(kB + 128, kB + 384),
                                         start=False, stop=False)
                        nc.tensor.matmul(psK3[hb][:], t[:, 128:256],
                                         mat(kR, kR + 256),
                                         start=False, stop=False)
                        nc.tensor.matmul(psK3[hb][:], t[:, 320:448],
                                         mat(kB, kB + 256),
                                         start=False, stop=True)

            # ---------------- projection ----------------
            res2 = cpool.tile([16, 2], F32, tag="res")
            SQUARE = mybir.ActivationFunctionType.Square
            for hb in range(2):
                v = psK3[hb][:].rearrange("p (ri c) -> p ri c",
                                          ri=2)[:, :, 0:16]
                sq = cpool.tile([128, 32], F32, tag=f"sq{hb}", name=f"sq{hb}")
                rs = cpool.tile([128, 1], F32, tag=f"rs{hb}", name=f"rs{hb}")
                nc.scalar.activation(
                    sq[:].rearrange("p (ri c) -> p ri c", ri=2), v, SQUARE,
                    accum_out=rs[:])
                psq = ppool.tile([16, 1], F32, tag="ps", name=f"q{hb}")
                nc.tensor.matmul(psq[:], t_pk32[:, 0:16], rs[:],
                                 start=True, stop=True)
                nc.vector.tensor_scalar(res2[:, hb:hb + 1], psq[:],
                                        t_pk32[0:16, 16:17], 0.5,
                                        op0=MULT, op1=ADD)
                nc.sync.dma_start(d_out[:, hb * HB:hb * HB + HB],
                                  res2[:, hb:hb + 1])

    nc.compile()
    return nc


# ---------------------------------------------------------------------------
# Entry point
# ---------------------------------------------------------------------------


def kernel(features, weights):
    global _PROGRAM
    from concourse.bass_utils import run_bass_kernel_spmd

    features = np.asarray(features)
    weights = np.asarray(weights)
    if _PROGRAM is None:
        _PROGRAM = _build_program()
    nc = _PROGRAM

    mats = _make_mats(weights)
    pk32 = _make_pk32()
    in_maps = []
    for c in range(NCORES):
        fc = features[c * B_CORE:(c + 1) * B_CORE]
        in_maps.append({
            "pk16": _make_pk16(fc),
            "pk32": pk32,
            "mats": mats,
        })

    last_err = None
    for attempt in range(3):
        try:
            res = run_bass_kernel_spmd(nc, in_maps, list(range(NCORES)))
            break
        except Exception as e:  # noqa: BLE001
            last_err = e
            import time

            time.sleep(10 * (attempt + 1))
    else:
        raise last_err
    out = np.concatenate([res.results[c]["out"][0] for c in range(NCORES)])
    return out.astype(np.float32)


if __name__ == "__main__":
    rng = np.random.default_rng(0)
    f = rng.standard_normal((256, 10)).astype(np.float32)
    w = (0.01 * rng.random((4, 10))).astype(np.float32)
    print(kernel(f, w)[:8])


# revision 12
# speedup vs baseline: 1.7608x; 1.1840x over previous
"""Trainium2 Bass kernel for the 14-wire quantum autoencoder swap test.

Math: wires 10-13 stay |0> until the swap test, so
P(aux=1) = 0.5 - 0.5*q with q = sum_{trash wires 7,8,9 = 0} |c_i|^2 of the
10-qubit state after AngleEmbedding + BasicEntanglerLayers.

All transforms run on the PE in fp16 (fp32 PSUM accumulation):
  state S~ [ft, w] per 16-sample half-batch, ft = par*64 + w0*32 + w1*16 + bl
  (par = w2 after C01,C12; w = wires 3..9 index, w9 = MSB).
  Per layer: stage G (RX0-2 + C01 + C12 [+ C90 of previous layer folded via
  row-permuted GkC90 blocks for the w9=1 column half]) flips to standard
  layout [w, ft']; stage K (RX3-9 + C34..C89, C23 via K2/K2b stationary split
  by par class) flips back. Zero-padded stationary columns keep every matmul
  writing all 128 PSUM partitions (fp32-family dst-partition ISA rule).
  Negations live in host-built blocks: [Xim_neg | Xre | Xim] per matrix.
Final: |.|^2 on w<16 cols, sel-matmul partition reduce, affine.
"""
import numpy as np

NCORES = 8
B_CORE = 32
HB = 16
DEPTH = 4
NQ = 10

# pk16 fp16 [128, 1088]: fsc 512 | pt 384 ([imn|re|im]) | zeros 192
C_FSC = 0          # per half: [Fre 128 | Fim 128] at rows hb*32+bl
C_PT = 512
C_ZERO = 896
PK16 = 1088

# pk32 f32 [128, 18]: sel [128,16], col16 = -0.5
PK32 = 18

# mats fp16 [128, 5760]: per layer: gR 384 | gC 384 (l>=1) | kR 384 | kB 384
L_COLS = [1152, 1536, 1536, 1536]
L_OFF = [0, 1152, 2688, 4224]
M_COLS = 5760

# ---------------------------------------------------------------------------
# Host-side constant construction
# ---------------------------------------------------------------------------

# ft class (par*4 + gg) -> g = w0*4 + w1*2 + w2
_FT_G = np.zeros(8, dtype=np.int64)
for _par in range(2):
    for _gg in range(4):
        _FT_G[_par * 4 + _gg] = (_gg >> 1) * 4 + (_gg & 1) * 2 + _par


def _perm_matrix(perm):
    m = np.zeros((len(perm), len(perm)))
    for src, dst in enumerate(perm):
        m[dst, src] = 1.0
    return m


def _cnot_chain_perm_p():
    perm = np.zeros(128, dtype=np.int64)
    for p in range(128):
        w = [(p >> k) & 1 for k in range(7)]
        for k in range(6):
            w[k + 1] ^= w[k]
        perm[p] = sum(w[k] << k for k in range(7))
    return perm


def _build_k2(weights_l):
    m = np.array([[1.0]], dtype=np.complex128)
    for w in (9, 8, 7, 6, 5, 4, 3):
        c, s = np.cos(weights_l[w] / 2), np.sin(weights_l[w] / 2)
        r = np.array([[c, -1j * s], [-1j * s, c]], dtype=np.complex128)
        m = np.kron(m, r)
    qa = _perm_matrix(_cnot_chain_perm_p())
    k2 = qa @ m
    k2b = k2 @ _perm_matrix(np.arange(128) ^ 1)
    return k2, k2b


def _build_gk(weights_l):
    m = np.array([[1.0]], dtype=np.complex128)
    for w in (0, 1, 2):
        c, s = np.cos(weights_l[w] / 2), np.sin(weights_l[w] / 2)
        r = np.array([[c, -1j * s], [-1j * s, c]], dtype=np.complex128)
        m = np.kron(m, r)
    perm = np.zeros(8, dtype=np.int64)
    for g in range(8):
        w0, w1, w2 = (g >> 2) & 1, (g >> 1) & 1, g & 1
        w1 ^= w0
        w2 ^= w1
        perm[g] = w0 * 4 + w1 * 2 + w2
    G = _perm_matrix(perm) @ m
    gk = np.zeros((128, 128), dtype=np.complex128)
    for ci in range(8):
        for co in range(8):
            v = G[_FT_G[co], _FT_G[ci]]
            if v != 0:
                for bl in range(HB):
                    gk[ci * 16 + bl, co * 16 + bl] = v
    return gk  # contraction: T[ft_out] = sum_ft_in S[ft_in] * gk[ft_in, ft_out]


def _blocks(m):
    """[im_neg | re | im] fp column triple for complex matrix m [128,128]."""
    return np.concatenate([-m.imag, m.real, m.imag], axis=1)


def _make_mats(weights):
    wt = weights.astype(np.float64).reshape(DEPTH, NQ)
    flip = np.arange(128) ^ 32
    cols = []
    for l in range(DEPTH):
        gk = _build_gk(wt[l])
        k2, k2b = _build_k2(wt[l])
        cols.append(_blocks(gk))
        if l >= 1:
            cols.append(_blocks(gk[flip, :]))
        cols.append(_blocks(k2.T))
        cols.append(_blocks(k2b.T))
    mats = np.concatenate(cols, axis=1)
    assert mats.shape == (128, M_COLS)
    return mats.astype(np.float16)


def _make_pk16(features_core):
    th = features_core.astype(np.float64)
    B = th.shape[0]
    c_emb, s_emb = np.cos(th / 2), np.sin(th / 2)
    v = np.stack([c_emb.astype(np.complex128), -1j * s_emb], axis=-1)

    pk = np.zeros((128, PK16), dtype=np.float64)
    # rows: sample b = hb*16+bl lives at partition hb*32+bl
    rows = np.array([(b // HB) * 32 + (b % HB) for b in range(B)])
    # pt: seed over wires 3..9: col j = w9*64+...+w3 (w3 = LSB)
    for j in range(128):
        val = np.ones(B, dtype=np.complex128)
        for k, w in enumerate((3, 4, 5, 6, 7, 8, 9)):
            val = val * v[:, w, (j >> k) & 1]
        pk[rows, C_PT + j] = -val.imag
        pk[rows, C_PT + 128 + j] = val.real
        pk[rows, C_PT + 256 + j] = val.imag
    # fsc: per half: Fre/Fim [16, 128]; col ft = cls*16+bl, row hb*32+bl
    for hb in range(2):
        for cls in range(8):
            g = _FT_G[cls]
            w0, w1, w2 = (g >> 2) & 1, (g >> 1) & 1, g & 1
            for bl in range(HB):
                b = hb * HB + bl
                F = v[b, 0, w0] * v[b, 1, w1] * v[b, 2, w2]
                col = C_FSC + hb * 256
                pk[hb * 32 + bl, col + cls * 16 + bl] = F.real
                pk[hb * 32 + bl, col + 128 + cls * 16 + bl] = F.imag
    return pk.astype(np.float16)


def _make_pk32():
    pk = np.zeros((128, PK32), dtype=np.float32)
    for ft in range(128):
        pk[ft, ft % 16] = 1.0
    pk[:, 16] = -0.5
    return pk


# ---------------------------------------------------------------------------
# Bass program
# ---------------------------------------------------------------------------

_PROGRAM = None


def _build_program():
    import concourse.bacc as bacc
    import concourse.mybir as mybir
    import concourse.tile as tile

    F32 = mybir.dt.float32
    F16 = mybir.dt.float16
    MULT = mybir.AluOpType.mult
    ADD = mybir.AluOpType.add

    nc = bacc.Bacc("TRN2", target_bir_lowering=False, debug=False,
                   num_devices=NCORES)

    d_pk16 = nc.dram_tensor("pk16", [128, PK16], F16, kind="ExternalInput")
    d_pk32 = nc.dram_tensor("pk32", [128, PK32], F32, kind="ExternalInput")
    d_mats = nc.dram_tensor("mats", [128, M_COLS], F16, kind="ExternalInput")
    d_out = nc.dram_tensor("out", [1, B_CORE], F32, kind="ExternalOutput")

    with tile.TileContext(nc) as tc:
        with (
            tc.tile_pool(name="const", bufs=1) as cpool,
            tc.tile_pool(name="psum", bufs=6, space="PSUM") as ppool,
            tc.tile_pool(name="psumb", bufs=2, space="PSUM") as bpool,
        ):
            t_pk16 = cpool.tile([128, PK16], F16, tag="pk16")
            t_pk32 = cpool.tile([128, PK32], F32, tag="pk32")
            t_mats = cpool.tile([128, M_COLS], F16, tag="mats")

            # DMAs in just-in-time order, single SP queue
            nc.sync.dma_start(t_pk16[0:64, 0:C_ZERO], d_pk16[0:64, 0:C_ZERO])
            nc.sync.dma_start(t_pk16[:, C_ZERO:PK16], d_pk16[:, C_ZERO:PK16])
            nc.sync.dma_start(t_mats[:, 0:384], d_mats[:, 0:384])
            nc.sync.dma_start(t_mats[:, 384:1152], d_mats[:, 384:1152])
            sl = slice(L_OFF[1], L_OFF[1] + L_COLS[1])
            nc.sync.dma_start(t_mats[:, sl], d_mats[:, sl])
            nc.sync.dma_start(t_pk32[:], d_pk32[:])
            for l in range(2, DEPTH):
                sl = slice(L_OFF[l], L_OFF[l] + L_COLS[l])
                nc.sync.dma_start(t_mats[:, sl], d_mats[:, sl])

            # PE warm-up: starts the PE ramp clock early
            t_wu = cpool.tile([128, 256], F16, tag="wu")
            nc.gpsimd.memset(t_wu[:], 0.0)
            ps_wu = ppool.tile([128, 128], F32, tag="ps", name="wu")
            for i in range(3):
                nc.tensor.matmul(ps_wu[:], t_wu[:, 0:128], t_wu[:, 0:128],
                                 start=True, stop=True)

            # state tiles (persistent; zero-padded layout [128, 512]:
            # data chunks at 0,128,256,384 (64 cols), Z at 64,192,320)
            def zfill(t):
                dst = t[:].rearrange("p (a b) -> p a b", a=4, b=128)
                src = t_pk16[:, C_ZERO:C_ZERO + 192].rearrange(
                    "p (a b) -> p a b", a=3, b=64)
                nc.gpsimd.tensor_copy(dst[:, 0:3, 64:128], src)

            s0 = []
            sAB = []
            tt = []
            for hb in range(2):
                s0.append(cpool.tile([128, 256], F16, tag=f"s0{hb}",
                                     name=f"s0{hb}"))
                a = cpool.tile([128, 512], F16, tag=f"sA{hb}", name=f"sA{hb}")
                b = cpool.tile([128, 512], F16, tag=f"sB{hb}", name=f"sB{hb}")
                t = cpool.tile([128, 512], F16, tag=f"tt{hb}", name=f"tt{hb}")
                sAB.append([a, b])
                tt.append(t)
            for hb in range(2):
                zfill(tt[hb])
            for hb in range(2):
                zfill(sAB[hb][0])
            for hb in range(2):
                zfill(sAB[hb][1])

            # chunk views of a zero-padded tile: [p, x(par/w9), y(ri), 64]
            def chunks(t):
                return t[:].rearrange("p (x y b) -> p x y b",
                                      x=2, y=2, b=128)[:, :, :, 0:64]

            # embedding matmuls (re/im split) -> psE -> s0
            psE = []
            for hb in range(2):
                r0 = hb * 32
                rows = slice(r0, r0 + HB)
                fre = t_pk16[r0:r0 + HB, C_FSC + hb * 256:C_FSC + hb * 256 + 128]
                fim = t_pk16[r0:r0 + HB,
                             C_FSC + hb * 256 + 128:C_FSC + hb * 256 + 256]
                ptimn = t_pk16[rows, C_PT:C_PT + 128]
                ptre = t_pk16[rows, C_PT + 128:C_PT + 256]
                ptim = t_pk16[rows, C_PT + 256:C_PT + 384]
                psEr = ppool.tile([128, 128], F32, tag="ps", name=f"per{hb}")
                psEi = ppool.tile([128, 128], F32, tag="ps", name=f"pei{hb}")
                nc.tensor.matmul(psEr[:], fre, ptre, start=True, stop=False)
                nc.tensor.matmul(psEr[:], fim, ptimn, start=False, stop=True)
                nc.tensor.matmul(psEi[:], fre, ptim, start=True, stop=False)
                nc.tensor.matmul(psEi[:], fim, ptre, start=False, stop=True)
                psE.append((psEr, psEi))
            for hb in range(2):
                nc.scalar.copy(s0[hb][:, 0:128], psE[hb][0][:])
                nc.vector.tensor_copy(s0[hb][:, 128:256], psE[hb][1][:])

            # ---------------- layers ----------------
            def mat(c0, c1):
                return t_mats[:, c0:c1]

            psK3 = [None, None]

            def emit_g(l, hb, gR, gC):
                pgr = ppool.tile([128, 128], F32, tag="ps",
                                 name=f"pgr{l}{hb}")
                pgi = ppool.tile([128, 128], F32, tag="ps",
                                 name=f"pgi{l}{hb}")
                if l == 0:
                    sre = s0[hb][:, 0:128]
                    sim = s0[hb][:, 128:256]
                    nc.tensor.matmul(pgr[:], sre, mat(gR + 128, gR + 256),
                                     start=True, stop=False)
                    nc.tensor.matmul(pgr[:], sim, mat(gR, gR + 128),
                                     start=False, stop=True)
                    nc.tensor.matmul(pgi[:], sre, mat(gR + 256, gR + 384),
                                     start=True, stop=False)
                    nc.tensor.matmul(pgi[:], sim, mat(gR + 128, gR + 256),
                                     start=False, stop=True)
                else:
                    sv = sAB[hb][(l - 1) % 2]
                    nc.tensor.matmul(pgr[:], sv[:, 0:128],
                                     mat(gR + 128, gR + 256),
                                     start=True, stop=False)
                    nc.tensor.matmul(pgr[:], sv[:, 192:320],
                                     mat(gC + 128, gC + 256),
                                     start=False, stop=False)
                    nc.tensor.matmul(pgr[:], sv[:, 128:256],
                                     mat(gR, gR + 128),
                                     start=False, stop=False)
                    nc.tensor.matmul(pgr[:], sv[:, 320:448],
                                     mat(gC, gC + 128),
                                     start=False, stop=True)
                    nc.tensor.matmul(pgi[:], sv[:, 0:128],
                                     mat(gR + 256, gR + 384),
                                     start=True, stop=False)
                    nc.tensor.matmul(pgi[:], sv[:, 192:320],
                                     mat(gC + 256, gC + 384),
                                     start=False, stop=False)
                    nc.tensor.matmul(pgi[:], sv[:, 128:256],
                                     mat(gR + 128, gR + 256),
                                     start=False, stop=False)
                    nc.tensor.matmul(pgi[:], sv[:, 320:448],
                                     mat(gC + 128, gC + 256),
                                     start=False, stop=True)
                return pgr, pgi

            def emit_k(l, hb, kR, kB):
                t = tt[hb]
                if l < DEPTH - 1:
                    pkr = ppool.tile([128, 128], F32, tag="ps",
                                     name=f"pkr{l}{hb}")
                    pki = ppool.tile([128, 128], F32, tag="ps",
                                     name=f"pki{l}{hb}")
                    nc.tensor.matmul(pkr[:], t[:, 0:128],
                                     mat(kR + 128, kR + 256),
                                     start=True, stop=False)
                    nc.tensor.matmul(pkr[:], t[:, 192:320],
                                     mat(kB + 128, kB + 256),
                                     start=False, stop=False)
                    nc.tensor.matmul(pkr[:], t[:, 128:256],
                                     mat(kR, kR + 128),
                                     start=False, stop=False)
                    nc.tensor.matmul(pkr[:], t[:, 320:448],
                                     mat(kB, kB + 128),
                                     start=False, stop=True)
                    nc.tensor.matmul(pki[:], t[:, 0:128],
                                     mat(kR + 256, kR + 384),
                                     start=True, stop=False)
                    nc.tensor.matmul(pki[:], t[:, 192:320],
                                     mat(kB + 256, kB + 384),
                                     start=False, stop=False)
                    nc.tensor.matmul(pki[:], t[:, 128:256],
                                     mat(kR + 128, kR + 256),
                                     start=False, stop=False)
                    nc.tensor.matmul(pki[:], t[:, 320:448],
                                     mat(kB + 128, kB + 256),
                                     start=False, stop=True)
                    return pkr, pki
                psK3[hb] = bpool.tile([128, 256], F32, tag="pb",
                                      name=f"pk3{hb}")
                nc.tensor.matmul(psK3[hb][:], t[:, 0:128],
                                 mat(kR + 128, kR + 384),
                                 start=True, stop=False)
                nc.tensor.matmul(psK3[hb][:], t[:, 192:320],
                                 mat(kB + 128, kB + 384),
                                 start=False, stop=False)
                nc.tensor.matmul(psK3[hb][:], t[:, 128:256],
                                 mat(kR, kR + 256),
                                 start=False, stop=False)
                nc.tensor.matmul(psK3[hb][:], t[:, 320:448],
                                 mat(kB, kB + 256),
                                 start=False, stop=True)
                return None

            for l in range(DEPTH):
                base = L_OFF[l]
                gR = base
                gC = base + 384
                kR = base + (768 if l >= 1 else 384)
                kB = kR + 384
                pg = [emit_g(l, hb, gR, gC) for hb in range(2)]
                for hb in range(2):
                    tch = chunks(tt[hb])
                    nc.scalar.copy(
                        tch[:, :, 0],
                        pg[hb][0][:].rearrange("p (par c) -> p par c", par=2))
                    nc.vector.tensor_copy(
                        tch[:, :, 1],
                        pg[hb][1][:].rearrange("p (par c) -> p par c", par=2))
                pk = [emit_k(l, hb, kR, kB) for hb in range(2)]
                if l < DEPTH - 1:
                    for hb in range(2):
                        sch = chunks(sAB[hb][l % 2])
                        nc.scalar.copy(
                            sch[:, :, 0],
                            pk[hb][0][:].rearrange("p (w9 c) -> p w9 c",
                                                   w9=2))
                        nc.vector.tensor_copy(
                            sch[:, :, 1],
                            pk[hb][1][:].rearrange("p (w9 c) -> p w9 c",
                                                   w9=2))

            # ---------------- projection ----------------
            res2 = cpool.tile([16, 2], F32, tag="res")
            SQUARE = mybir.ActivationFunctionType.Square
            for hb in range(2):
                v = psK3[hb][:].rearrange("p (ri c) -> p ri c",
                                          ri=2)[:, :, 0:16]
                sq = cpool.tile([128, 32], F32, tag=f"sq{hb}", name=f"sq{hb}")
                rs = cpool.tile([128, 1], F32, tag=f"rs{hb}", name=f"rs{hb}")
                nc.scalar.activation(
                    sq[:].rearrange("p (ri c) -> p ri c", ri=2), v, SQUARE,
                    accum_out=rs[:])
                psq = ppool.tile([16, 1], F32, tag="ps", name=f"q{hb}")
                nc.tensor.matmul(psq[:], t_pk32[:, 0:16], rs[:],
                                 start=True, stop=True)
                nc.vector.tensor_scalar(res2[:, hb:hb + 1], psq[:],
                                        t_pk32[0:16, 16:17], 0.5,
                                        op0=MULT, op1=ADD)
                nc.sync.dma_start(d_out[:, hb * HB:hb * HB + HB],
                                  res2[:, hb:hb + 1])

    nc.compile()
    return nc


# ---------------------------------------------------------------------------
# Entry point
# ---------------------------------------------------------------------------


def kernel(features, weights):
    global _PROGRAM
    from concourse.bass_utils import run_bass_kernel_spmd

    features = np.asarray(features)
    weights = np.asarray(weights)
    if _PROGRAM is None:
        _PROGRAM = _build_program()
    nc = _PROGRAM

    mats = _make_mats(weights)
    pk32 = _make_pk32()
    in_maps = []
    for c in range(NCORES):
        fc = features[c * B_CORE:(c + 1) * B_CORE]
        in_maps.append({
            "pk16": _make_pk16(fc),
            "pk32": pk32,
            "mats": mats,
        })

    last_err = None
    for attempt in range(3):
        try:
            res = run_bass_kernel_spmd(nc, in_maps, list(range(NCORES)))
            break
        except Exception as e:  # noqa: BLE001
            last_err = e
            import time

            time.sleep(10 * (attempt + 1))
    else:
        raise last_err
    out = np.concatenate([res.results[c]["out"][0] for c in range(NCORES)])
    return out.astype(np.float32)


if __name__ == "__main__":
    rng = np.random.default_rng(0)
    f = rng.standard_normal((256, 10)).astype(np.float32)
    w = (0.01 * rng.random((4, 10))).astype(np.float32)
    print(kernel(f, w)[:8])
